# revision 1
# baseline (speedup 1.0000x reference)
"""AttentionReadout kernel for 8 trn2 NeuronCores.

Problem: gate-MLP attention readout over 500k nodes, D=256, G=1024 graphs.
    h = tanh(x @ W1 + b1); s = h @ W2 + b2
    attn = segment_softmax(s, batch); out[g] = sum_{n in g} attn[n] * x[n]

Strategy (data-parallel over graphs, SPMD single program):
  - 1024 graphs split evenly: 128 graphs per core; node ranges padded to a
    common length so one program serves all 8 cores.
  - Segment softmax without max-subtraction (scores are O(3), exp safe in f32)
    so attn = e / denom with e = exp(s); both segment sums are computed by a
    single PE matmul against a one-hot graph-membership matrix scaled by e:
        P[g, 0:256] += onehot_e.T @ x_chunk ;  P[g, 256] += onehot_e.T @ ones
    accumulated in PSUM over all chunks, normalized once at the end.
  - Scores per 128-node chunk: PE transpose x -> xT, u^T = W1^T @ xT (fp32r),
    h^T = tanh(u^T + b1) (bf16), s_col = (h^T block).T @ W2, e = exp(s + b2).
  - onehot_e built on DVE in one fused op: (iota == B_col) * e_col, where B is
    a host-built per-node local-graph-id table (-1 marks padding).
"""

import sys

sys.path.insert(0, "/opt/trn_rl_repo")

from contextlib import ExitStack

import numpy as np
import ml_dtypes

import concourse.bass as bass
import concourse.tile as tile
from concourse import mybir
from concourse.bass_utils import run_bass_kernel_spmd

N_NODES = 500_000
D = 256
G = 1024
N_CORES = 8
GPC = G // N_CORES  # 128 graphs per core
CHUNK = 128  # nodes per pooling matmul (contraction dim)
GROUP = 4  # chunks per inner group (512 nodes)
DC = D + 2  # x row + ones column + pad (even N for fp32r matmul)


def _split_waits(nc, max_waits=1):
    """Hoist extra semaphore waits onto preceding same-engine NOPs.

    The walrus build in this container rejects instructions carrying more
    than one embedded sync wait ("Too many sync wait commands"); engines
    execute their stream in order, so a wait on a preceding NOP is
    equivalent.
    """
    n = 0
    for fn in nc.m.functions:
        for blk in fn.blocks:
            newlist = []
            for ins in blk.instructions:
                si = ins.sync_info
                if si is not None and len(si.on_wait) > max_waits:
                    waits = list(si.on_wait)
                    keep, extra = waits[:max_waits], waits[max_waits:]
                    for w in extra:
                        n += 1
                        nop = mybir.InstNoOp(
                            name=f"waitsplit-{n}-{ins.name}", ins=[], outs=[]
                        )
                        nop.engine = ins.engine
                        nop.sync_info = mybir.SyncInfo(on_wait=[w], on_update=[])
                        nc.register_instruction(nop, overwrite=True)
                        newlist.append(nop)
                    ins.sync_info = mybir.SyncInfo(
                        on_wait=keep, on_update=list(si.on_update)
                    )
                newlist.append(ins)
            blk.instructions[:] = newlist
    return n


def build_nc(n_chunks, split=True):
    """Build the SPMD program for one core processing n_chunks*128 nodes."""
    f32 = mybir.dt.float32
    f32r = mybir.dt.float32r
    bf16 = mybir.dt.bfloat16
    n_groups = n_chunks // GROUP
    assert n_chunks % GROUP == 0

    nc = bass.Bass()
    x_d = nc.declare_dram_parameter("x", [n_chunks * CHUNK, DC], f32r, isOutput=False)
    b_d = nc.declare_dram_parameter("bid", [CHUNK, n_chunks], f32, isOutput=False)
    iota_d = nc.declare_dram_parameter("iota", [CHUNK, GPC], f32, isOutput=False)
    ident_d = nc.declare_dram_parameter("ident", [CHUNK, CHUNK], f32r, isOutput=False)
    w1_d = nc.declare_dram_parameter("W1", [D, D], f32r, isOutput=False)
    b1_d = nc.declare_dram_parameter("b1", [D, 1], f32, isOutput=False)
    w2_d = nc.declare_dram_parameter("W2", [D, 1], bf16, isOutput=False)
    b2_d = nc.declare_dram_parameter("b2", [CHUNK, 1], f32, isOutput=False)
    out_d = nc.declare_dram_parameter("out", [GPC, D], f32, isOutput=True)

    x_v = x_d.rearrange("(c p) d -> p c d", p=CHUNK)  # [128, n_chunks, 257]

    with tile.TileContext(nc) as tc, ExitStack() as ctx:
        const = ctx.enter_context(tc.tile_pool(name="const", bufs=1))
        xpool = ctx.enter_context(tc.tile_pool(name="x", bufs=8))
        hpool = ctx.enter_context(tc.tile_pool(name="h", bufs=3))
        epool = ctx.enter_context(tc.tile_pool(name="e", bufs=4))
        ohpool = ctx.enter_context(tc.tile_pool(name="oh", bufs=4))
        opool = ctx.enter_context(tc.tile_pool(name="o", bufs=1))
        ps_xt = ctx.enter_context(tc.tile_pool(name="ps_xt", bufs=2, space="PSUM"))
        ps_u = ctx.enter_context(tc.tile_pool(name="ps_u", bufs=1, space="PSUM"))
        ps_s = ctx.enter_context(tc.tile_pool(name="ps_s", bufs=1, space="PSUM"))
        ps_p = ctx.enter_context(tc.tile_pool(name="ps_p", bufs=1, space="PSUM"))

        # Resident constants
        bid_t = const.tile([CHUNK, n_chunks], f32, tag="bid", name="bid_t")
        nc.sync.dma_start(bid_t[:], b_d[:])
        iota_t = const.tile([CHUNK, GPC], f32, tag="iota", name="iota_t")
        nc.sync.dma_start(iota_t[:], iota_d[:])
        ident_t = const.tile([CHUNK, CHUNK], f32r, tag="ident", name="ident_t")
        nc.sync.dma_start(ident_t[:], ident_d[:])
        w1_t = []
        for i in range(2):
            for j in range(2):
                t = const.tile([CHUNK, CHUNK], f32r, tag=f"w1_{i}{j}", name=f"w1_{i}{j}")
                nc.sync.dma_start(
                    t[:], w1_d[i * CHUNK : (i + 1) * CHUNK, j * CHUNK : (j + 1) * CHUNK]
                )
                w1_t.append(t)
        b1_t = const.tile([CHUNK, 2], f32, tag="b1", name="b1_t")
        nc.sync.dma_start(b1_t[:], b1_d.rearrange("(h p) o -> p (h o)", p=CHUNK))
        w2_t = const.tile([CHUNK, 2], bf16, tag="w2", name="w2_t")
        nc.sync.dma_start(w2_t[:], w2_d.rearrange("(h p) o -> p (h o)", p=CHUNK))
        b2_t = const.tile([CHUNK, 1], f32, tag="b2", name="b2_t")
        nc.sync.dma_start(b2_t[:], b2_d[:])

        # Persistent pooled accumulator [128 graphs, 257] in PSUM
        p_acc = ps_p.tile([GPC, DC], f32, tag="p_acc", name="p_acc")

        for gi in range(n_groups):
            c0 = gi * GROUP
            # one DMA: 512 nodes -> [128, GROUP, 257]
            xw = xpool.tile([CHUNK, GROUP, DC], f32r, tag="xw", name="xw")
            nc.sync.dma_start(xw[:], x_v[:, c0 : c0 + GROUP, :])

            # transposes: x chunks -> xT [d, nodes] (two 128-halves of d)
            xt_ps = [
                ps_xt.tile([CHUNK, GROUP * CHUNK], f32r, tag=f"xt{h}", name=f"xt{h}") for h in range(2)
            ]
            for h in range(2):
                for j in range(GROUP):
                    nc.tensor.transpose(
                        xt_ps[h][:, j * CHUNK : (j + 1) * CHUNK],
                        xw[:, j, h * CHUNK : (h + 1) * CHUNK],
                        ident_t[:],
                    )
            xt_sb = [
                xpool.tile([CHUNK, GROUP * CHUNK], f32r, tag=f"xtsb{h}", name=f"xtsb{h}")
                for h in range(2)
            ]
            nc.scalar.copy(xt_sb[0][:], xt_ps[0][:])
            nc.vector.tensor_copy(xt_sb[1][:], xt_ps[1][:])

            # u^T = W1^T @ xT  (two d_out halves, accumulate over d_in halves)
            hT = [
                hpool.tile([CHUNK, GROUP * CHUNK], bf16, tag=f"hT{h}", name=f"hT{h}") for h in range(2)
            ]
            for h in range(2):
                u_ps = ps_u.tile([CHUNK, GROUP * CHUNK], f32, tag=f"u{h}", name=f"u{h}")
                for k in range(2):
                    nc.tensor.matmul(
                        u_ps[:],
                        w1_t[2 * k + h][:],
                        xt_sb[k][:],
                        start=(k == 0),
                        stop=(k == 1),
                    )
                # h^T = tanh(u + b1) -> bf16
                nc.scalar.activation(
                    hT[h][:],
                    u_ps[:],
                    mybir.ActivationFunctionType.Tanh,
                    bias=b1_t[:, h : h + 1],
                )

            # scores as columns: s[128,j] = sum_h (hT_h block j).T @ W2_h
            s_ps = ps_s.tile([CHUNK, GROUP], f32, tag="s", name="s_ps")
            for j in range(GROUP):
                for h in range(2):
                    nc.tensor.matmul(
                        s_ps[:, j : j + 1],
                        hT[h][:, j * CHUNK : (j + 1) * CHUNK],
                        w2_t[:, h : h + 1],
                        start=(j == 0 and h == 0),
                        stop=(j == GROUP - 1 and h == 1),
                    )
            e_t = epool.tile([CHUNK, GROUP], f32, tag="e", name="e_t")
            nc.scalar.activation(
                e_t[:], s_ps[:], mybir.ActivationFunctionType.Exp, bias=b2_t[:, 0:1]
            )

            # pooling: P += onehot_e.T @ [x | 1]
            for j in range(GROUP):
                c = c0 + j
                oh = ohpool.tile([CHUNK, GPC], f32r, tag="oh", name="oh")
                nc.vector.tensor_scalar(
                    oh[:],
                    iota_t[:],
                    bid_t[:, c : c + 1],
                    e_t[:, j : j + 1],
                    mybir.AluOpType.is_equal,
                    mybir.AluOpType.mult,
                )
                nc.tensor.matmul(
                    p_acc[:],
                    oh[:],
                    xw[:, j, :],
                    start=(c == 0),
                    stop=(c == n_chunks - 1),
                )

        # epilogue: out = P[:, 0:256] / max(P[:, 256], tiny)
        denom = opool.tile([GPC, 1], f32, tag="denom", name="denom")
        nc.vector.tensor_scalar_max(denom[:], p_acc[:, D : D + 1], 1e-30)
        rec = opool.tile([GPC, 1], f32, tag="rec", name="rec")
        nc.vector.reciprocal(rec[:], denom[:])
        out_t = opool.tile([GPC, D], f32, tag="out", name="out_t")
        nc.vector.tensor_scalar(
            out_t[:], p_acc[:, 0:D], rec[:], None, mybir.AluOpType.mult
        )
        nc.sync.dma_start(out_d[:], out_t[:])

    if split:
        _split_waits(nc)
    return nc


def prepare_inputs(x, batch, W1, b1, W2, b2):
    """Host-side sharding: per-core padded x (+ones col), local graph ids."""
    x = np.asarray(x, dtype=np.float32)
    batch = np.asarray(batch).astype(np.int64)
    starts = np.searchsorted(batch, np.arange(G + 1))
    core_bounds = starts[:: GPC][: N_CORES + 1]
    counts = np.diff(core_bounds)
    tile_nodes = GROUP * CHUNK
    npad = int(-(-counts.max() // tile_nodes) * tile_nodes)
    n_chunks = npad // CHUNK

    iota = np.broadcast_to(
        np.arange(GPC, dtype=np.float32), (CHUNK, GPC)
    ).copy()
    ident = np.eye(CHUNK, dtype=np.float32)
    W1 = np.asarray(W1, dtype=np.float32)
    b1 = np.asarray(b1, dtype=np.float32).reshape(D, 1)
    W2 = np.asarray(W2, dtype=np.float32).reshape(D, 1).astype(ml_dtypes.bfloat16)
    b2 = np.full((CHUNK, 1), np.float32(np.asarray(b2).reshape(())), dtype=np.float32)

    in_maps = []
    for c in range(N_CORES):
        r0, r1 = int(core_bounds[c]), int(core_bounds[c + 1])
        n = r1 - r0
        xc = np.zeros((npad, DC), dtype=np.float32)
        xc[:n, :D] = x[r0:r1]
        xc[:, D] = 1.0  # ones column -> denominator; col D+1 stays 0 (pad)
        bc = np.full(npad, -1.0, dtype=np.float32)
        bc[:n] = (batch[r0:r1] - c * GPC).astype(np.float32)
        bc2d = np.ascontiguousarray(bc.reshape(n_chunks, CHUNK).T)  # [128, n_chunks]
        in_maps.append(
            {
                "x": xc,
                "bid": bc2d,
                "iota": iota,
                "ident": ident,
                "W1": W1,
                "b1": b1,
                "W2": W2,
                "b2": b2,
            }
        )
    return in_maps, n_chunks


def kernel(x, batch, num_graphs, W1, b1, W2, b2):
    assert int(num_graphs) == G
    in_maps, n_chunks = prepare_inputs(x, batch, W1, b1, W2, b2)
    nc = build_nc(n_chunks)
    res = run_bass_kernel_spmd(nc, in_maps, list(range(N_CORES)))
    out = np.concatenate([res.results[c]["out"] for c in range(N_CORES)], axis=0)
    return out.astype(np.float32)

